# revision 9
# baseline (speedup 1.0000x reference)
"""Multi-head causal attention with RoPE on 8 Trainium2 cores.

Sharding: batch (2) x head-groups (4 heads each) -> 8 shards, one per core.

Per core, pipelined over 512-query chunks (tch = 0..3):
  QKV(tch):  qT/kT feature-major [(4x64), 512] = w.T @ x.T chunk (+bias, bf16),
             v token-major [4x(128, 4 heads, 64+ones)] (bf16)
  RoPE(tch): partition-swap DMA + 3 bf16 DVE ops per q/k tile chunk
  ATTN(tch): per head-pair (partitions 0-63 / 64-127 of a ct tile), S^T
             matmuls run row-tiled CONCURRENTLY on the PE (K=64 each);
             j-blocks processed in pairs sharing a [128,1024] 2-bank PSUM
             tile so exp batches 2 blocks per activation; P = exp(S/8) in
             bf16; PV accumulated per head with a ones-column giving the
             softmax denominator Z in PSUM row 64; normalize via DVE
             reciprocal + gpsimd partition_broadcast.
  OUT(tch):  out_partial^T [E, 512] = w_out_rows.T @ oTn chunk (fp32r),
             PSUM drained on DVE, DMA to HBM.
Host sums the 4 head-group partials per batch and adds biases.

bf16 everywhere in QKV/attention (fp32 PSUM accumulation); out-proj fp32r.
Chunks are emitted so the Tile list-scheduler overlaps ACT-bound attention
with PE-bound QKV of the next chunk.
"""
import numpy as np

B, T, E, H = 2, 2048, 1024, 16
D = 64
HPC = 4           # heads per core
CG = HPC * D      # 256 channels per shard
NE = E // 128     # 8 contraction chunks
NJ = T // 128     # 16 key tiles
NCH = T // 512    # 4 query chunks
ROPE_BASE = 10000.0

_CACHE = {}


def _bf16():
    import ml_dtypes
    return ml_dtypes.bfloat16


def _host_constants():
    bf16 = _bf16()
    t = np.arange(T, dtype=np.float32)
    inv_freq = (1.0 / (ROPE_BASE ** (np.arange(0, D, 2, dtype=np.float32) / D))).astype(np.float32)
    freqs = t[:, None] * inv_freq[None, :]          # [T, 32]
    fcos = np.cos(freqs).T.astype(np.float32)       # [32, T]
    fsin = np.sin(freqs).T.astype(np.float32)
    cosT = np.vstack([fcos, fcos])                  # [64, T]
    sinnT = np.vstack([-fsin, fsin])                # [64, T] sign-folded for rotate_half
    cos2 = np.ascontiguousarray(np.vstack([cosT, cosT])).astype(bf16)    # [128, T]
    sinn2 = np.ascontiguousarray(np.vstack([sinnT, sinnT])).astype(bf16)
    mask = np.triu(np.ones((128, 128), dtype=np.float32)).astype(bf16)   # valid: q_local >= k_local
    return cos2, sinn2, mask


def _build(repeat=1):
    import concourse.bacc as bacc
    import concourse.mybir as mybir
    import concourse.tile as tile

    F32 = mybir.dt.float32
    F32R = mybir.dt.float32r
    BF16 = mybir.dt.bfloat16
    AF = mybir.ActivationFunctionType

    nc = bacc.Bacc("TRN2", target_bir_lowering=False, debug=False, enable_asserts=True)

    xT = nc.dram_tensor("xT", [E, T], BF16, kind="ExternalInput").ap()
    wq = nc.dram_tensor("wq", [E, CG], BF16, kind="ExternalInput").ap()
    wk = nc.dram_tensor("wk", [E, CG], BF16, kind="ExternalInput").ap()
    wv = nc.dram_tensor("wv", [E, CG], BF16, kind="ExternalInput").ap()
    wo = nc.dram_tensor("wo", [CG, E], F32R, kind="ExternalInput").ap()
    cos2 = nc.dram_tensor("cos2", [128, T], BF16, kind="ExternalInput").ap()
    sinn2 = nc.dram_tensor("sinn2", [128, T], BF16, kind="ExternalInput").ap()
    mask = nc.dram_tensor("mask", [128, 128], BF16, kind="ExternalInput").ap()
    bq = nc.dram_tensor("bq", [CG], F32, kind="ExternalInput").ap()
    bk = nc.dram_tensor("bk", [CG], F32, kind="ExternalInput").ap()
    outT = nc.dram_tensor("outT", [E, T], F32, kind="ExternalOutput").ap()

    with tile.TileContext(nc) as tc:
        with tc.tile_pool(name="persist", bufs=1) as pp:
            q_t = [pp.tile([128, T], BF16, tag=f"q{i}", name=f"q{i}") for i in range(2)]
            k_t = [pp.tile([128, T], BF16, tag=f"k{i}", name=f"k{i}") for i in range(2)]
            v_t = [pp.tile([128, HPC, D + 1], BF16, tag=f"v{j}", name=f"v{j}") for j in range(NJ)]
            oTn = [pp.tile([128, T], F32R, tag=f"o{i}", name=f"o{i}") for i in range(2)]
            wo_sb = [pp.tile([128, E], F32R, tag=f"wo{i}", name=f"wosb{i}") for i in range(2)]
            xts = pp.tile([128, NE, T], BF16, tag="xts", name="xts")
            wq_sb = pp.tile([128, NE, CG], BF16, tag="wq", name="wqsb")
            wk_sb = pp.tile([128, NE, CG], BF16, tag="wk", name="wksb")
            wv_sb = pp.tile([128, NE, CG], BF16, tag="wv", name="wvsb")
            cos_sb = pp.tile([128, T], BF16, tag="cos")
            sinn_sb = pp.tile([128, T], BF16, tag="sinn")
            mask_sb = pp.tile([128, 128], BF16, tag="mask")
            bq_sb = pp.tile([128, 2], F32, tag="bq")
            bk_sb = pp.tile([128, 2], F32, tag="bk")
            warm = pp.tile([1, 8], F32, tag="warm")

            import contextlib
            rep_ctx = (tc.For_i(0, repeat, 1,
                                hint_engines=(mybir.EngineType.PE,
                                              mybir.EngineType.DVE,
                                              mybir.EngineType.Activation,
                                              mybir.EngineType.SP,
                                              mybir.EngineType.Pool))
                       if repeat > 1 else contextlib.nullcontext())
            with rep_ctx:
                with tc.tile_pool(name="rope", bufs=4) as rp, \
                     tc.tile_pool(name="ppool", bufs=4) as ap_, \
                     tc.tile_pool(name="norm", bufs=4) as np_, \
                     tc.tile_pool(name="outb", bufs=2) as op_, \
                     tc.tile_pool(name="s_psum", bufs=2, space="PSUM") as sp, \
                     tc.tile_pool(name="qkv_psum", bufs=2, space="PSUM") as qpp, \
                     tc.tile_pool(name="pv_psum", bufs=2, space="PSUM") as pvp:

                    # preload the exp activation table while DMA warms up
                    nc.vector.memset(warm, 0.0)
                    nc.scalar.activation(out=warm, in_=warm, func=AF.Exp)

                    # ---- DMAs, batched, first-needed-first ----
                    xTr = xT.rearrange("(a p) t -> p a t", p=128)
                    nc.sync.dma_start(out=xts[:, :, 0:512], in_=xTr[:, :, 0:512])
                    nc.scalar.dma_start(out=wq_sb, in_=wq.rearrange("(a p) c -> p a c", p=128))
                    nc.scalar.dma_start(out=wk_sb, in_=wk.rearrange("(a p) c -> p a c", p=128))
                    nc.scalar.dma_start(out=wv_sb, in_=wv.rearrange("(a p) c -> p a c", p=128))
                    nc.scalar.dma_start(out=bq_sb, in_=bq.rearrange("(a p) -> p a", p=128))
                    nc.scalar.dma_start(out=bk_sb, in_=bk.rearrange("(a p) -> p a", p=128))
                    nc.scalar.dma_start(out=mask_sb, in_=mask)
                    nc.scalar.dma_start(out=cos_sb, in_=cos2)
                    nc.scalar.dma_start(out=sinn_sb, in_=sinn2)
                    for tchl in range(1, NCH):
                        csl = slice(512 * tchl, 512 * (tchl + 1))
                        nc.sync.dma_start(out=xts[:, :, csl], in_=xTr[:, :, csl])
                    for i in range(2):
                        nc.scalar.dma_start(out=wo_sb[i], in_=wo[128 * i:128 * (i + 1), :])

                    def qkv_chunk(tch):
                        csl = slice(512 * tch, 512 * (tch + 1))
                        for w_sb, bias_sb, dst in ((wq_sb, bq_sb, q_t), (wk_sb, bk_sb, k_t)):
                            for ct in range(2):
                                ps = qpp.tile([128, 512], F32, tag="qkv", name="psqk")
                                for e in range(NE):
                                    nc.tensor.matmul(
                                        ps,
                                        lhsT=w_sb[:, e, 128 * ct:128 * (ct + 1)],
                                        rhs=xts[:, e, csl],
                                        start=(e == 0), stop=(e == NE - 1),
                                    )
                                nc.scalar.activation(
                                    out=dst[ct][:, csl], in_=ps,
                                    func=AF.Identity, bias=bias_sb[:, ct:ct + 1],
                                )
                        for j in range(4 * tch, 4 * tch + 4):
                            ps = qpp.tile([128, 512], F32, tag="qkv", name="psv")
                            for e in range(NE):
                                nc.tensor.matmul(
                                    ps[:, 0:CG],
                                    lhsT=xts[:, e, 128 * j:128 * (j + 1)],
                                    rhs=wv_sb[:, e, :],
                                    start=(e == 0), stop=(e == NE - 1),
                                )
                            nc.vector.tensor_copy(
                                out=v_t[j][:, :, 0:D],
                                in_=ps[:, 0:CG].rearrange("p (h d) -> p h d", h=HPC),
                            )
                            nc.gpsimd.memset(v_t[j][:, :, D:D + 1], 1.0)

                    def rope_chunk(tch):
                        csl = slice(512 * tch, 512 * (tch + 1))
                        for t_ in (q_t[0], k_t[0], q_t[1], k_t[1]):
                            swq = rp.tile([128, 512], BF16, tag="swq", name="swq")
                            for hh in (0, 64):
                                nc.scalar.dma_start(out=swq[hh:hh + 32, :], in_=t_[hh + 32:hh + 64, csl])
                                nc.scalar.dma_start(out=swq[hh + 32:hh + 64, :], in_=t_[hh:hh + 32, csl])
                            nc.vector.tensor_mul(out=swq, in0=swq, in1=sinn_sb[:, csl])
                            nc.vector.tensor_mul(out=t_[:, csl], in0=t_[:, csl], in1=cos_sb[:, csl])
                            nc.vector.tensor_add(out=t_[:, csl], in0=t_[:, csl], in1=swq)

                    def attention_chunk(tch):
                        i0 = 512 * tch
                        nj = 4 * (tch + 1)
                        for ct in range(2):
                            pvs = [pvp.tile([128, 512], F32, tag="pv", name=f"pv{h}")
                                   for h in (0, 1)]
                            for ja in range(0, nj, 2):
                                jb = ja + 1
                                pinfo = {}
                                for h in (0, 1):
                                    poff = 64 * h
                                    st = sp.tile([128, 1024], F32, tag="s", name="s")
                                    off = 0
                                    offs = {}
                                    for j in (ja, jb):
                                        w = min(512, i0 + 512 - 128 * j)
                                        c0 = max(i0, 128 * j)
                                        if off % 512 != 0 and (off % 512) + w > 512:
                                            off = (off // 512 + 1) * 512
                                        nc.tensor.matmul(
                                            st[:, off:off + w],
                                            lhsT=k_t[ct][poff:poff + 64, 128 * j:128 * j + 128],
                                            rhs=q_t[ct][poff:poff + 64, c0:i0 + 512],
                                            start=True, stop=True,
                                        )
                                        offs[j] = (off, w, c0)
                                        off += w
                                    pt = ap_.tile([128, 1024], BF16, tag="p", name="p")
                                    nc.scalar.activation(out=pt[:, 0:off], in_=st[:, 0:off],
                                                         func=AF.Exp, scale=0.125)
                                    for j in (ja, jb):
                                        o, w, c0 = offs[j]
                                        if 128 * j >= i0:
                                            nc.vector.tensor_mul(out=pt[:, o:o + 128],
                                                                 in0=pt[:, o:o + 128], in1=mask_sb)
                                    pinfo[h] = (pt, offs)
                                for h in (0, 1):
                                    pt, offs = pinfo[h]
                                    for j in (ja, jb):
                                        o, w, c0 = offs[j]
                                        nc.tensor.matmul(
                                            pvs[h][0:D + 1, c0 - i0:512],
                                            lhsT=v_t[j][:, 2 * ct + h, :],
                                            rhs=pt[:, o:o + w],
                                            start=(j == 0), stop=(j == nj - 1),
                                            skip_group_check=True,
                                        )
                            for h in (0, 1):
                                rz = np_.tile([1, 512], F32, tag="rz")
                                nc.vector.reciprocal(out=rz, in_=pvs[h][D:D + 1, :])
                                bc = np_.tile([64, 512], F32, tag="bc")
                                nc.gpsimd.partition_broadcast(bc, rz)
                                nc.vector.tensor_mul(
                                    out=oTn[ct][64 * h:64 * h + 64, i0:i0 + 512],
                                    in0=pvs[h][0:D, :], in1=bc,
                                )

                    outTr = outT.rearrange("(a p) t -> p a t", p=128)

                    def outproj_chunk(tch):
                        csl = slice(512 * tch, 512 * (tch + 1))
                        ob = op_.tile([128, NE, 512], F32, tag="ob")
                        for et in range(NE):
                            ps = qpp.tile([128, 512], F32, tag="qkv", name="psop")
                            for cc in range(2):
                                nc.tensor.matmul(
                                    ps,
                                    lhsT=wo_sb[cc][:, 128 * et:128 * (et + 1)],
                                    rhs=oTn[cc][:, csl],
                                    start=(cc == 0), stop=(cc == 1),
                                )
                            nc.vector.tensor_copy(out=ob[:, et, :], in_=ps)
                        nc.sync.dma_start(out=outTr[:, :, csl], in_=ob)

                    qkv_chunk(0)
                    rope_chunk(0)
                    for tch in range(NCH):
                        attention_chunk(tch)
                        if tch + 1 < NCH:
                            qkv_chunk(tch + 1)
                            rope_chunk(tch + 1)
                        outproj_chunk(tch)

    nc.compile()
    return nc


def get_nc(repeat=1):
    key = f"nc{repeat}"
    if key not in _CACHE:
        _CACHE[key] = _build(repeat)
    return _CACHE[key]


def make_in_maps(x, w_qkv, b_qkv):
    bf16 = _bf16()
    cos2, sinn2, mask = _host_constants()
    x = np.asarray(x, dtype=np.float32)
    w_qkv = np.asarray(w_qkv, dtype=np.float32)
    b_qkv = np.asarray(b_qkv, dtype=np.float32)
    in_maps = []
    for c in range(8):
        b, hg = divmod(c, 4)
        sl = slice(CG * hg, CG * (hg + 1))
        in_maps.append({
            "xT": np.ascontiguousarray(x[b].T).astype(bf16),
            "wq": np.ascontiguousarray(w_qkv[:, 0 * E:1 * E][:, sl]).astype(bf16),
            "wk": np.ascontiguousarray(w_qkv[:, 1 * E:2 * E][:, sl]).astype(bf16),
            "wv": np.ascontiguousarray(w_qkv[:, 2 * E:3 * E][:, sl]).astype(bf16),
            "wo": None,  # filled by caller (needs w_out)
            "cos2": cos2, "sinn2": sinn2, "mask": mask,
            "bq": np.ascontiguousarray(b_qkv[0 * E:1 * E][sl]),
            "bk": np.ascontiguousarray(b_qkv[1 * E:2 * E][sl]),
        })
    return in_maps


def kernel(x, w_qkv, b_qkv, w_out, b_out, _res_out=None):
    from concourse.bass_utils import run_bass_kernel_spmd

    x = np.asarray(x, dtype=np.float32)
    w_qkv = np.asarray(w_qkv, dtype=np.float32)
    b_qkv = np.asarray(b_qkv, dtype=np.float32)
    w_out = np.asarray(w_out, dtype=np.float32)
    b_out = np.asarray(b_out, dtype=np.float32)

    nc = get_nc()
    in_maps = make_in_maps(x, w_qkv, b_qkv)
    for c in range(8):
        hg = c % 4
        in_maps[c]["wo"] = np.ascontiguousarray(w_out[CG * hg:CG * (hg + 1), :])

    res = run_bass_kernel_spmd(nc, in_maps, list(range(8)))
    if _res_out is not None:
        _res_out.append(res)

    out = np.empty((B, T, E), np.float32)
    for b in range(B):
        acc = res.results[4 * b + 0]["outT"].astype(np.float64)
        for g in range(1, 4):
            acc += res.results[4 * b + g]["outT"]
        out[b] = acc.T
    bias = b_qkv[2 * E:3 * E].astype(np.float64) @ w_out.astype(np.float64) + b_out
    out += bias.astype(np.float32)[None, None, :]
    return out


# revision 10
# speedup vs baseline: 1.0215x; 1.0215x over previous
"""Multi-head causal attention with RoPE on 8 Trainium2 cores.

Sharding: batch (2) x head-groups (4 heads each) -> 8 shards, one per core.

Per core, pipelined over 512-query chunks (tch = 0..3):
  QKV(tch):  qT/kT feature-major [(4x64), 512] = w.T @ x.T chunk (+bias, bf16),
             v token-major [4x(128, 4 heads, 64+ones)] (bf16)
  RoPE(tch): partition-swap DMA + 3 bf16 DVE ops per q/k tile chunk
  ATTN(tch): per head-pair (partitions 0-63 / 64-127 of a ct tile), S^T
             matmuls run row-tiled CONCURRENTLY on the PE (K=64 each);
             j-blocks processed in pairs sharing a [128,1024] 2-bank PSUM
             tile so exp batches 2 blocks per activation; P = exp(S/8) in
             bf16; PV accumulated per head with a ones-column giving the
             softmax denominator Z in PSUM row 64; normalize via DVE
             reciprocal + gpsimd partition_broadcast.
  OUT(tch):  out_partial^T [E, 512] = w_out_rows.T @ oTn chunk (fp32r),
             PSUM drained on DVE, DMA to HBM.
Host sums the 4 head-group partials per batch and adds biases.

bf16 everywhere in QKV/attention (fp32 PSUM accumulation); out-proj fp32r.
Chunks are emitted so the Tile list-scheduler overlaps ACT-bound attention
with PE-bound QKV of the next chunk.
"""
import numpy as np

B, T, E, H = 2, 2048, 1024, 16
D = 64
HPC = 4           # heads per core
CG = HPC * D      # 256 channels per shard
NE = E // 128     # 8 contraction chunks
NJ = T // 128     # 16 key tiles
NCH = T // 512    # 4 query chunks
ROPE_BASE = 10000.0

_CACHE = {}


def _bf16():
    import ml_dtypes
    return ml_dtypes.bfloat16


def _host_constants():
    bf16 = _bf16()
    t = np.arange(T, dtype=np.float32)
    inv_freq = (1.0 / (ROPE_BASE ** (np.arange(0, D, 2, dtype=np.float32) / D))).astype(np.float32)
    freqs = t[:, None] * inv_freq[None, :]          # [T, 32]
    fcos = np.cos(freqs).T.astype(np.float32)       # [32, T]
    fsin = np.sin(freqs).T.astype(np.float32)
    cosT = np.vstack([fcos, fcos])                  # [64, T]
    sinnT = np.vstack([-fsin, fsin])                # [64, T] sign-folded for rotate_half
    cos2 = np.ascontiguousarray(np.vstack([cosT, cosT])).astype(bf16)    # [128, T]
    sinn2 = np.ascontiguousarray(np.vstack([sinnT, sinnT])).astype(bf16)
    mask = np.triu(np.ones((128, 128), dtype=np.float32)).astype(bf16)   # valid: q_local >= k_local
    return cos2, sinn2, mask


def _build(repeat=1):
    import concourse.bacc as bacc
    import concourse.mybir as mybir
    import concourse.tile as tile

    F32 = mybir.dt.float32
    F32R = mybir.dt.float32r
    BF16 = mybir.dt.bfloat16
    AF = mybir.ActivationFunctionType

    nc = bacc.Bacc("TRN2", target_bir_lowering=False, debug=False, enable_asserts=True)

    xT = nc.dram_tensor("xT", [E, T], BF16, kind="ExternalInput").ap()
    wq = nc.dram_tensor("wq", [E, CG], BF16, kind="ExternalInput").ap()
    wk = nc.dram_tensor("wk", [E, CG], BF16, kind="ExternalInput").ap()
    wv = nc.dram_tensor("wv", [E, CG], BF16, kind="ExternalInput").ap()
    wo = nc.dram_tensor("wo", [CG, E], F32R, kind="ExternalInput").ap()
    cos2 = nc.dram_tensor("cos2", [128, T], BF16, kind="ExternalInput").ap()
    sinn2 = nc.dram_tensor("sinn2", [128, T], BF16, kind="ExternalInput").ap()
    mask = nc.dram_tensor("mask", [128, 128], BF16, kind="ExternalInput").ap()
    bq = nc.dram_tensor("bq", [CG], F32, kind="ExternalInput").ap()
    bk = nc.dram_tensor("bk", [CG], F32, kind="ExternalInput").ap()
    outT = nc.dram_tensor("outT", [E, T], F32, kind="ExternalOutput").ap()

    with tile.TileContext(nc) as tc:
        with tc.tile_pool(name="persist", bufs=1) as pp:
            q_t = [pp.tile([128, T], BF16, tag=f"q{i}", name=f"q{i}") for i in range(2)]
            k_t = [pp.tile([128, T], BF16, tag=f"k{i}", name=f"k{i}") for i in range(2)]
            v_t = [pp.tile([128, HPC, D + 1], BF16, tag=f"v{j}", name=f"v{j}") for j in range(NJ)]
            oTn = [pp.tile([128, T], F32R, tag=f"o{i}", name=f"o{i}") for i in range(2)]
            wo_sb = [pp.tile([128, E], F32R, tag=f"wo{i}", name=f"wosb{i}") for i in range(2)]
            xts = pp.tile([128, NE, T], BF16, tag="xts", name="xts")
            wq_sb = pp.tile([128, NE, CG], BF16, tag="wq", name="wqsb")
            wk_sb = pp.tile([128, NE, CG], BF16, tag="wk", name="wksb")
            wv_sb = pp.tile([128, NE, CG], BF16, tag="wv", name="wvsb")
            cos_sb = pp.tile([128, T], BF16, tag="cos")
            sinn_sb = pp.tile([128, T], BF16, tag="sinn")
            mask_sb = pp.tile([128, 128], BF16, tag="mask")
            bq_sb = pp.tile([128, 2], F32, tag="bq")
            bk_sb = pp.tile([128, 2], F32, tag="bk")
            warm = pp.tile([1, 8], F32, tag="warm")

            for _rep in range(repeat):
                with tc.tile_pool(name="rope", bufs=4) as rp, \
                     tc.tile_pool(name="ppool", bufs=4) as ap_, \
                     tc.tile_pool(name="norm", bufs=4) as np_, \
                     tc.tile_pool(name="outb", bufs=2) as op_, \
                     tc.tile_pool(name="s_psum", bufs=2, space="PSUM") as sp, \
                     tc.tile_pool(name="qkv_psum", bufs=2, space="PSUM") as qpp, \
                     tc.tile_pool(name="pv_psum", bufs=2, space="PSUM") as pvp:

                    # preload the exp activation table while DMA warms up
                    nc.vector.memset(warm, 0.0)
                    nc.scalar.activation(out=warm, in_=warm, func=AF.Exp)

                    # ---- DMAs, batched, first-needed-first ----
                    xTr = xT.rearrange("(a p) t -> p a t", p=128)
                    nc.sync.dma_start(out=xts[:, :, 0:512], in_=xTr[:, :, 0:512])
                    nc.scalar.dma_start(out=wq_sb, in_=wq.rearrange("(a p) c -> p a c", p=128))
                    nc.scalar.dma_start(out=wk_sb, in_=wk.rearrange("(a p) c -> p a c", p=128))
                    nc.scalar.dma_start(out=wv_sb, in_=wv.rearrange("(a p) c -> p a c", p=128))
                    nc.scalar.dma_start(out=bq_sb, in_=bq.rearrange("(a p) -> p a", p=128))
                    nc.scalar.dma_start(out=bk_sb, in_=bk.rearrange("(a p) -> p a", p=128))
                    nc.scalar.dma_start(out=mask_sb, in_=mask)
                    nc.scalar.dma_start(out=cos_sb, in_=cos2)
                    nc.scalar.dma_start(out=sinn_sb, in_=sinn2)
                    for tchl in range(1, NCH):
                        csl = slice(512 * tchl, 512 * (tchl + 1))
                        nc.sync.dma_start(out=xts[:, :, csl], in_=xTr[:, :, csl])
                    for i in range(2):
                        nc.scalar.dma_start(out=wo_sb[i], in_=wo[128 * i:128 * (i + 1), :])

                    def qkv_chunk(tch):
                        csl = slice(512 * tch, 512 * (tch + 1))
                        for w_sb, bias_sb, dst in ((wq_sb, bq_sb, q_t), (wk_sb, bk_sb, k_t)):
                            for ct in range(2):
                                ps = qpp.tile([128, 512], F32, tag="qkv", name="psqk")
                                for e in range(NE):
                                    nc.tensor.matmul(
                                        ps,
                                        lhsT=w_sb[:, e, 128 * ct:128 * (ct + 1)],
                                        rhs=xts[:, e, csl],
                                        start=(e == 0), stop=(e == NE - 1),
                                    )
                                nc.scalar.activation(
                                    out=dst[ct][:, csl], in_=ps,
                                    func=AF.Identity, bias=bias_sb[:, ct:ct + 1],
                                )
                        for j in range(4 * tch, 4 * tch + 4):
                            ps = qpp.tile([128, 512], F32, tag="qkv", name="psv")
                            for e in range(NE):
                                nc.tensor.matmul(
                                    ps[:, 0:CG],
                                    lhsT=xts[:, e, 128 * j:128 * (j + 1)],
                                    rhs=wv_sb[:, e, :],
                                    start=(e == 0), stop=(e == NE - 1),
                                )
                            nc.vector.tensor_copy(
                                out=v_t[j][:, :, 0:D],
                                in_=ps[:, 0:CG].rearrange("p (h d) -> p h d", h=HPC),
                            )
                            nc.gpsimd.memset(v_t[j][:, :, D:D + 1], 1.0)

                    def rope_chunk(tch):
                        csl = slice(512 * tch, 512 * (tch + 1))
                        for t_ in (q_t[0], k_t[0], q_t[1], k_t[1]):
                            swq = rp.tile([128, 512], BF16, tag="swq", name="swq")
                            for hh in (0, 64):
                                nc.scalar.dma_start(out=swq[hh:hh + 32, :], in_=t_[hh + 32:hh + 64, csl])
                                nc.scalar.dma_start(out=swq[hh + 32:hh + 64, :], in_=t_[hh:hh + 32, csl])
                            nc.vector.tensor_mul(out=swq, in0=swq, in1=sinn_sb[:, csl])
                            nc.vector.tensor_mul(out=t_[:, csl], in0=t_[:, csl], in1=cos_sb[:, csl])
                            nc.vector.tensor_add(out=t_[:, csl], in0=t_[:, csl], in1=swq)

                    def attention_chunk(tch):
                        i0 = 512 * tch
                        nj = 4 * (tch + 1)
                        for ct in range(2):
                            pvs = [pvp.tile([128, 512], F32, tag="pv", name=f"pv{h}")
                                   for h in (0, 1)]
                            for ja in range(0, nj, 2):
                                jb = ja + 1
                                pinfo = {}
                                for h in (0, 1):
                                    poff = 64 * h
                                    st = sp.tile([128, 1024], F32, tag="s", name="s")
                                    off = 0
                                    offs = {}
                                    for j in (ja, jb):
                                        w = min(512, i0 + 512 - 128 * j)
                                        c0 = max(i0, 128 * j)
                                        if off % 512 != 0 and (off % 512) + w > 512:
                                            off = (off // 512 + 1) * 512
                                        nc.tensor.matmul(
                                            st[:, off:off + w],
                                            lhsT=k_t[ct][poff:poff + 64, 128 * j:128 * j + 128],
                                            rhs=q_t[ct][poff:poff + 64, c0:i0 + 512],
                                            start=True, stop=True,
                                        )
                                        offs[j] = (off, w, c0)
                                        off += w
                                    pt = ap_.tile([128, 1024], BF16, tag="p", name="p")
                                    nc.scalar.activation(out=pt[:, 0:off], in_=st[:, 0:off],
                                                         func=AF.Exp, scale=0.125)
                                    for j in (ja, jb):
                                        o, w, c0 = offs[j]
                                        if 128 * j >= i0:
                                            nc.vector.tensor_mul(out=pt[:, o:o + 128],
                                                                 in0=pt[:, o:o + 128], in1=mask_sb)
                                    pinfo[h] = (pt, offs)
                                for h in (0, 1):
                                    pt, offs = pinfo[h]
                                    for j in (ja, jb):
                                        o, w, c0 = offs[j]
                                        nc.tensor.matmul(
                                            pvs[h][0:D + 1, c0 - i0:512],
                                            lhsT=v_t[j][:, 2 * ct + h, :],
                                            rhs=pt[:, o:o + w],
                                            start=(j == 0), stop=(j == nj - 1),
                                            skip_group_check=True,
                                        )
                            for h in (0, 1):
                                rz = np_.tile([1, 512], F32, tag="rz")
                                nc.vector.reciprocal(out=rz, in_=pvs[h][D:D + 1, :])
                                bc = np_.tile([64, 512], F32, tag="bc")
                                nc.gpsimd.partition_broadcast(bc, rz)
                                nc.vector.tensor_mul(
                                    out=oTn[ct][64 * h:64 * h + 64, i0:i0 + 512],
                                    in0=pvs[h][0:D, :], in1=bc,
                                )

                    outTr = outT.rearrange("(a p) t -> p a t", p=128)

                    def outproj_chunk(tch):
                        csl = slice(512 * tch, 512 * (tch + 1))
                        ob = op_.tile([128, NE, 512], F32, tag="ob")
                        for et in range(NE):
                            ps = qpp.tile([128, 512], F32, tag="qkv", name="psop")
                            for cc in range(2):
                                nc.tensor.matmul(
                                    ps,
                                    lhsT=wo_sb[cc][:, 128 * et:128 * (et + 1)],
                                    rhs=oTn[cc][:, csl],
                                    start=(cc == 0), stop=(cc == 1),
                                )
                            nc.vector.tensor_copy(out=ob[:, et, :], in_=ps)
                        nc.sync.dma_start(out=outTr[:, :, csl], in_=ob)

                    qkv_chunk(0)
                    rope_chunk(0)
                    for tch in range(NCH):
                        attention_chunk(tch)
                        if tch + 1 < NCH:
                            qkv_chunk(tch + 1)
                            rope_chunk(tch + 1)
                        outproj_chunk(tch)

    nc.compile()
    return nc


def get_nc(repeat=1):
    key = f"nc{repeat}"
    if key not in _CACHE:
        _CACHE[key] = _build(repeat)
    return _CACHE[key]


def make_in_maps(x, w_qkv, b_qkv):
    bf16 = _bf16()
    cos2, sinn2, mask = _host_constants()
    x = np.asarray(x, dtype=np.float32)
    w_qkv = np.asarray(w_qkv, dtype=np.float32)
    b_qkv = np.asarray(b_qkv, dtype=np.float32)
    in_maps = []
    for c in range(8):
        b, hg = divmod(c, 4)
        sl = slice(CG * hg, CG * (hg + 1))
        in_maps.append({
            "xT": np.ascontiguousarray(x[b].T).astype(bf16),
            "wq": np.ascontiguousarray(w_qkv[:, 0 * E:1 * E][:, sl]).astype(bf16),
            "wk": np.ascontiguousarray(w_qkv[:, 1 * E:2 * E][:, sl]).astype(bf16),
            "wv": np.ascontiguousarray(w_qkv[:, 2 * E:3 * E][:, sl]).astype(bf16),
            "wo": None,  # filled by caller (needs w_out)
            "cos2": cos2, "sinn2": sinn2, "mask": mask,
            "bq": np.ascontiguousarray(b_qkv[0 * E:1 * E][sl]),
            "bk": np.ascontiguousarray(b_qkv[1 * E:2 * E][sl]),
        })
    return in_maps


def kernel(x, w_qkv, b_qkv, w_out, b_out, _res_out=None):
    from concourse.bass_utils import run_bass_kernel_spmd

    x = np.asarray(x, dtype=np.float32)
    w_qkv = np.asarray(w_qkv, dtype=np.float32)
    b_qkv = np.asarray(b_qkv, dtype=np.float32)
    w_out = np.asarray(w_out, dtype=np.float32)
    b_out = np.asarray(b_out, dtype=np.float32)

    nc = get_nc()
    in_maps = make_in_maps(x, w_qkv, b_qkv)
    for c in range(8):
        hg = c % 4
        in_maps[c]["wo"] = np.ascontiguousarray(w_out[CG * hg:CG * (hg + 1), :])

    res = run_bass_kernel_spmd(nc, in_maps, list(range(8)))
    if _res_out is not None:
        _res_out.append(res)

    out = np.empty((B, T, E), np.float32)
    for b in range(B):
        acc = res.results[4 * b + 0]["outT"].astype(np.float64)
        for g in range(1, 4):
            acc += res.results[4 * b + g]["outT"]
        out[b] = acc.T
    bias = b_qkv[2 * E:3 * E].astype(np.float64) @ w_out.astype(np.float64) + b_out
    out += bias.astype(np.float32)[None, None, :]
    return out


# revision 18
# speedup vs baseline: 3.2368x; 3.1688x over previous
"""Multi-head causal attention with RoPE on 8 Trainium2 cores.

Sharding: batch (2) x head-groups (4 heads each) -> 8 shards, one per core.

Per core, pipelined over 512-query chunks (tch = 0..3):
  QKV(tch):  qT/kT feature-major [(4x64), 512] = w.T @ x.T chunk (+bias, bf16),
             v token-major [4x(128, 4 heads, 64+ones)] (bf16)
  RoPE(tch): partition-swap DMA + 3 bf16 DVE ops per q/k tile chunk
  ATTN(tch): per head-pair (partitions 0-63 / 64-127 of a ct tile), S^T
             matmuls run row-tiled CONCURRENTLY on the PE (K=64 each);
             j-blocks processed in pairs sharing a [128,1024] 2-bank PSUM
             tile so exp batches 2 blocks per activation; P = exp(S/8) in
             bf16; PV accumulated per head with a ones-column giving the
             softmax denominator Z in PSUM row 64; normalize via DVE
             reciprocal + gpsimd partition_broadcast.
  OUT(tch):  out_partial^T [E, 512] = w_out_rows.T @ oTn chunk (fp32r),
             PSUM drained on DVE, DMA to HBM.
Host sums the 4 head-group partials per batch and adds biases.

bf16 everywhere in QKV/attention (fp32 PSUM accumulation); out-proj fp32r.
Chunks are emitted so the Tile list-scheduler overlaps ACT-bound attention
with PE-bound QKV of the next chunk.
"""
import numpy as np

B, T, E, H = 2, 2048, 1024, 16
D = 64
HPC = 4           # heads per core
CG = HPC * D      # 256 channels per shard
NE = E // 128     # 8 contraction chunks
NJ = T // 128     # 16 key tiles
NCH = T // 512    # 4 query chunks
ROPE_BASE = 10000.0

_CACHE = {}


def _bf16():
    import ml_dtypes
    return ml_dtypes.bfloat16


def _host_constants():
    bf16 = _bf16()
    t = np.arange(T, dtype=np.float32)
    inv_freq = (1.0 / (ROPE_BASE ** (np.arange(0, D, 2, dtype=np.float32) / D))).astype(np.float32)
    freqs = t[:, None] * inv_freq[None, :]          # [T, 32]
    fcos = np.cos(freqs).T.astype(np.float32)       # [32, T]
    fsin = np.sin(freqs).T.astype(np.float32)
    cosT = np.vstack([fcos, fcos])                  # [64, T]
    sinnT = np.vstack([-fsin, fsin])                # [64, T] sign-folded for rotate_half
    cos2 = np.ascontiguousarray(np.vstack([cosT, cosT])).astype(bf16)    # [128, T]
    sinn2 = np.ascontiguousarray(np.vstack([sinnT, sinnT])).astype(bf16)
    mask = np.triu(np.ones((128, 128), dtype=np.float32)).astype(bf16)   # valid: q_local >= k_local
    return cos2, sinn2, mask


def _build(repeat=1):
    import concourse.bacc as bacc
    import concourse.mybir as mybir
    import concourse.tile as tile

    F32 = mybir.dt.float32
    F32R = mybir.dt.float32r
    BF16 = mybir.dt.bfloat16
    AF = mybir.ActivationFunctionType

    nc = bacc.Bacc("TRN2", target_bir_lowering=False, debug=False, enable_asserts=True)

    xT = nc.dram_tensor("xT", [E, T], BF16, kind="ExternalInput").ap()
    wq = nc.dram_tensor("wq", [E, CG], BF16, kind="ExternalInput").ap()
    wk = nc.dram_tensor("wk", [E, CG], BF16, kind="ExternalInput").ap()
    wv = nc.dram_tensor("wv", [E, CG], BF16, kind="ExternalInput").ap()
    wo = nc.dram_tensor("wo", [CG, E], F32R, kind="ExternalInput").ap()
    cos2 = nc.dram_tensor("cos2", [128, T], BF16, kind="ExternalInput").ap()
    sinn2 = nc.dram_tensor("sinn2", [128, T], BF16, kind="ExternalInput").ap()
    mask = nc.dram_tensor("mask", [128, 128], BF16, kind="ExternalInput").ap()
    bq = nc.dram_tensor("bq", [CG], F32, kind="ExternalInput").ap()
    bk = nc.dram_tensor("bk", [CG], F32, kind="ExternalInput").ap()
    outT = nc.dram_tensor("outT", [E, T], F32, kind="ExternalOutput").ap()

    with tile.TileContext(nc) as tc:
        with tc.tile_pool(name="persist", bufs=1) as pp:
            q_t = [pp.tile([128, T], BF16, tag=f"q{i}", name=f"q{i}") for i in range(2)]
            k_t = [pp.tile([128, T], BF16, tag=f"k{i}", name=f"k{i}") for i in range(2)]
            v_t = [pp.tile([128, HPC, D + 1], BF16, tag=f"v{j}", name=f"v{j}") for j in range(NJ)]
            oTn = [pp.tile([128, T], F32R, tag=f"o{i}", name=f"o{i}") for i in range(2)]
            wo_sb = [pp.tile([128, E], F32R, tag=f"wo{i}", name=f"wosb{i}") for i in range(2)]
            xts = pp.tile([128, NE, T], BF16, tag="xts", name="xts")
            wq_sb = pp.tile([128, NE, CG], BF16, tag="wq", name="wqsb")
            wk_sb = pp.tile([128, NE, CG], BF16, tag="wk", name="wksb")
            wv_sb = pp.tile([128, NE, CG], BF16, tag="wv", name="wvsb")
            cos_sb = pp.tile([128, T], BF16, tag="cos")
            sinn_sb = pp.tile([128, T], BF16, tag="sinn")
            mask_sb = pp.tile([128, 128], BF16, tag="mask")
            bq_sb = pp.tile([128, 2], F32, tag="bq")
            bk_sb = pp.tile([128, 2], F32, tag="bk")
            warm = pp.tile([1, 8], F32, tag="warm")

            for _rep in range(repeat):
                with tc.tile_pool(name="rope", bufs=6) as rp, \
                     tc.tile_pool(name="ppool", bufs=6) as ap_, \
                     tc.tile_pool(name="norm", bufs=8) as np_, \
                     tc.tile_pool(name="outb", bufs=2) as op_, \
                     tc.tile_pool(name="s_psum", bufs=2, space="PSUM") as sp, \
                     tc.tile_pool(name="qkv_psum", bufs=2, space="PSUM") as qpp, \
                     tc.tile_pool(name="pv_psum", bufs=2, space="PSUM") as pvp:

                    # preload the exp activation table while DMA warms up
                    nc.vector.memset(warm, 0.0)
                    nc.scalar.activation(out=warm, in_=warm, func=AF.Exp)

                    # ---- DMAs, batched, first-needed-first ----
                    xTr = xT.rearrange("(a p) t -> p a t", p=128)
                    nc.sync.dma_start(out=xts[:, :, 0:512], in_=xTr[:, :, 0:512])
                    nc.scalar.dma_start(out=wq_sb, in_=wq.rearrange("(a p) c -> p a c", p=128))
                    nc.scalar.dma_start(out=wk_sb, in_=wk.rearrange("(a p) c -> p a c", p=128))
                    nc.scalar.dma_start(out=wv_sb, in_=wv.rearrange("(a p) c -> p a c", p=128))
                    nc.scalar.dma_start(out=bq_sb, in_=bq.rearrange("(a p) -> p a", p=128))
                    nc.scalar.dma_start(out=bk_sb, in_=bk.rearrange("(a p) -> p a", p=128))
                    nc.scalar.dma_start(out=mask_sb, in_=mask)
                    nc.scalar.dma_start(out=cos_sb, in_=cos2)
                    nc.scalar.dma_start(out=sinn_sb, in_=sinn2)
                    for tchl in range(1, NCH):
                        csl = slice(512 * tchl, 512 * (tchl + 1))
                        nc.sync.dma_start(out=xts[:, :, csl], in_=xTr[:, :, csl])
                    for i in range(2):
                        nc.scalar.dma_start(out=wo_sb[i], in_=wo[128 * i:128 * (i + 1), :])

                    def qk_chunk(tch):
                        csl = slice(512 * tch, 512 * (tch + 1))
                        for w_sb, bias_sb, dst in ((wq_sb, bq_sb, q_t), (wk_sb, bk_sb, k_t)):
                            for ct in range(2):
                                ps = qpp.tile([128, 512], F32, tag="qkv", name="psqk")
                                for e in range(NE):
                                    nc.tensor.matmul(
                                        ps,
                                        lhsT=w_sb[:, e, 128 * ct:128 * (ct + 1)],
                                        rhs=xts[:, e, csl],
                                        start=(e == 0), stop=(e == NE - 1),
                                    )
                                nc.vector.tensor_scalar_add(
                                    out=dst[ct][:, csl], in0=ps,
                                    scalar1=bias_sb[:, ct:ct + 1],
                                )
                    def v_chunk(tch):
                        for j in range(4 * tch, 4 * tch + 4):
                            ps = qpp.tile([128, 512], F32, tag="qkv", name="psv")
                            for e in range(NE):
                                nc.tensor.matmul(
                                    ps[:, 0:CG],
                                    lhsT=xts[:, e, 128 * j:128 * (j + 1)],
                                    rhs=wv_sb[:, e, :],
                                    start=(e == 0), stop=(e == NE - 1),
                                )
                            nc.vector.tensor_copy(
                                out=v_t[j][:, :, 0:D],
                                in_=ps[:, 0:CG].rearrange("p (h d) -> p h d", h=HPC),
                            )
                            nc.gpsimd.memset(v_t[j][:, :, D:D + 1], 1.0)

                    def rope_chunk(tch):
                        csl = slice(512 * tch, 512 * (tch + 1))
                        for t_ in (q_t[0], k_t[0], q_t[1], k_t[1]):
                            swq = rp.tile([128, 512], BF16, tag="swq", name="swq")
                            for hh in (0, 64):
                                nc.scalar.dma_start(out=swq[hh:hh + 32, :], in_=t_[hh + 32:hh + 64, csl])
                                nc.scalar.dma_start(out=swq[hh + 32:hh + 64, :], in_=t_[hh:hh + 32, csl])
                            nc.vector.tensor_mul(out=swq, in0=swq, in1=sinn_sb[:, csl])
                            nc.vector.tensor_mul(out=t_[:, csl], in0=t_[:, csl], in1=cos_sb[:, csl])
                            nc.vector.tensor_add(out=t_[:, csl], in0=t_[:, csl], in1=swq)

                    def _emit_pv(ct, pvs, pinfo, nj, i0):
                        for h in (0, 1):
                            pt, offs = pinfo[h]
                            for j, (o, w, c0) in sorted(offs.items()):
                                nc.tensor.matmul(
                                    pvs[h][0:D + 1, c0 - i0:512],
                                    lhsT=v_t[j][:, 2 * ct + h, :],
                                    rhs=pt[:, o:o + w],
                                    start=(j == 0), stop=(j == nj - 1),
                                    skip_group_check=True,
                                )

                    def attention_chunk(tch):
                        i0 = 512 * tch
                        nj = 4 * (tch + 1)
                        for ct in range(2):
                            pvs = [pvp.tile([128, 512], F32, tag="pv", name=f"pv{h}")
                                   for h in (0, 1)]
                            prev = None
                            for ja in range(0, nj, 2):
                                jb = ja + 1
                                pinfo = {}
                                for h in (0, 1):
                                    poff = 64 * h
                                    st = sp.tile([128, 1024], F32, tag="s", name="s")
                                    off = 0
                                    offs = {}
                                    for j in (ja, jb):
                                        w = min(512, i0 + 512 - 128 * j)
                                        c0 = max(i0, 128 * j)
                                        if off % 512 != 0 and (off % 512) + w > 512:
                                            off = (off // 512 + 1) * 512
                                        nc.tensor.matmul(
                                            st[:, off:off + w],
                                            lhsT=k_t[ct][poff:poff + 64, 128 * j:128 * j + 128],
                                            rhs=q_t[ct][poff:poff + 64, c0:i0 + 512],
                                            start=True, stop=True,
                                        )
                                        offs[j] = (off, w, c0)
                                        off += w
                                    pt = ap_.tile([128, 1024], BF16, tag="p", name="p")
                                    nc.scalar.activation(out=pt[:, 0:off], in_=st[:, 0:off],
                                                         func=AF.Exp, scale=0.125)
                                    for j in (ja, jb):
                                        o, w, c0 = offs[j]
                                        if 128 * j >= i0:
                                            nc.vector.tensor_mul(out=pt[:, o:o + 128],
                                                                 in0=pt[:, o:o + 128], in1=mask_sb)
                                    pinfo[h] = (pt, offs)
                                if prev is not None:
                                    _emit_pv(ct, pvs, prev, nj, i0)
                                prev = pinfo
                            _emit_pv(ct, pvs, prev, nj, i0)
                            for h in (0, 1):
                                rz = np_.tile([1, 512], F32, tag="rz")
                                nc.vector.reciprocal(out=rz, in_=pvs[h][D:D + 1, :])
                                bc = np_.tile([64, 512], F32, tag="bc")
                                nc.gpsimd.partition_broadcast(bc, rz)
                                nc.vector.tensor_mul(
                                    out=oTn[ct][64 * h:64 * h + 64, i0:i0 + 512],
                                    in0=pvs[h][0:D, :], in1=bc,
                                )

                    outTr = outT.rearrange("(a p) t -> p a t", p=128)

                    def outproj_chunk(tch, drain="vector"):
                        csl = slice(512 * tch, 512 * (tch + 1))
                        ob = op_.tile([128, NE, 512], F32, tag="ob")
                        for et in range(NE):
                            ps = qpp.tile([128, 512], F32, tag="qkv", name="psop")
                            for cc in range(2):
                                nc.tensor.matmul(
                                    ps,
                                    lhsT=wo_sb[cc][:, 128 * et:128 * (et + 1)],
                                    rhs=oTn[cc][:, csl],
                                    start=(cc == 0), stop=(cc == 1),
                                )
                            if drain == "scalar":
                                nc.scalar.copy(out=ob[:, et, :], in_=ps)
                            else:
                                nc.vector.tensor_copy(out=ob[:, et, :], in_=ps)
                        nc.sync.dma_start(out=outTr[:, :, csl], in_=ob)

                    qk_chunk(0)
                    v_chunk(0)
                    rope_chunk(0)
                    for tch in range(NCH):
                        attention_chunk(tch)
                        if tch + 1 < NCH:
                            qk_chunk(tch + 1)
                            v_chunk(tch + 1)
                            rope_chunk(tch + 1)
                        if tch == 0:
                            outproj_chunk(0)
                    for tch in range(1, NCH):
                        outproj_chunk(tch, drain="scalar" if tch == 3 else "vector")

    nc.compile()
    return nc


def get_nc(repeat=1):
    key = f"nc{repeat}"
    if key not in _CACHE:
        _CACHE[key] = _build(repeat)
    return _CACHE[key]


def make_in_maps(x, w_qkv, b_qkv):
    bf16 = _bf16()
    cos2, sinn2, mask = _host_constants()
    x = np.asarray(x, dtype=np.float32)
    w_qkv = np.asarray(w_qkv, dtype=np.float32)
    b_qkv = np.asarray(b_qkv, dtype=np.float32)
    in_maps = []
    for c in range(8):
        b, hg = divmod(c, 4)
        sl = slice(CG * hg, CG * (hg + 1))
        in_maps.append({
            "xT": np.ascontiguousarray(x[b].T).astype(bf16),
            "wq": np.ascontiguousarray(w_qkv[:, 0 * E:1 * E][:, sl]).astype(bf16),
            "wk": np.ascontiguousarray(w_qkv[:, 1 * E:2 * E][:, sl]).astype(bf16),
            "wv": np.ascontiguousarray(w_qkv[:, 2 * E:3 * E][:, sl]).astype(bf16),
            "wo": None,  # filled by caller (needs w_out)
            "cos2": cos2, "sinn2": sinn2, "mask": mask,
            "bq": np.ascontiguousarray(b_qkv[0 * E:1 * E][sl]),
            "bk": np.ascontiguousarray(b_qkv[1 * E:2 * E][sl]),
        })
    return in_maps


def kernel(x, w_qkv, b_qkv, w_out, b_out, _res_out=None):
    from concourse.bass_utils import run_bass_kernel_spmd

    x = np.asarray(x, dtype=np.float32)
    w_qkv = np.asarray(w_qkv, dtype=np.float32)
    b_qkv = np.asarray(b_qkv, dtype=np.float32)
    w_out = np.asarray(w_out, dtype=np.float32)
    b_out = np.asarray(b_out, dtype=np.float32)

    nc = get_nc()
    in_maps = make_in_maps(x, w_qkv, b_qkv)
    for c in range(8):
        hg = c % 4
        in_maps[c]["wo"] = np.ascontiguousarray(w_out[CG * hg:CG * (hg + 1), :])

    res = run_bass_kernel_spmd(nc, in_maps, list(range(8)))
    if _res_out is not None:
        _res_out.append(res)

    out = np.empty((B, T, E), np.float32)
    for b in range(B):
        acc = res.results[4 * b + 0]["outT"].astype(np.float64)
        for g in range(1, 4):
            acc += res.results[4 * b + g]["outT"]
        out[b] = acc.T
    bias = b_qkv[2 * E:3 * E].astype(np.float64) @ w_out.astype(np.float64) + b_out
    out += bias.astype(np.float32)[None, None, :]
    return out
